# revision 1
# baseline (speedup 1.0000x reference)
"""BinaryConv2D Trainium2 kernel.

Reference computation:
    out = conv2d(sign(x), sign(w), SAME, stride 1)   # sign(v) = +1 if v>=0 else -1
    x: (64, 56, 56, 128) f32, w: (3, 3, 128, 256) f32 -> out (64, 56, 56, 256) f32

Strategy (data-parallel over batch, 8 images per NeuronCore):
  1. SWDGE cast-DMA x f32 -> bf16 (HBM->HBM), 2 images per DMA.  The cast
     preserves sign, and only the sign bit is consumed downstream.
  2. Per image pair: HW xbar DMA-transpose (DRAM->SBUF) [6272 px, 128 ch] ->
     [128 ch, 6272 px] bf16.  Weights are binarized host-side and loaded with
     another xbar transpose.
  3. One DVE tensor_scalar op per image binarizes via bit ops on the bf16
     pattern ((v & 0x8000) | 0x3F80 -> exactly +-1.0) while scattering rows
     into a zero-padded 58x58 layout (SAME padding becomes pointer shifts).
  4. 3x3 conv = 9 accumulating matmuls per output tile.  Output stays
     pixel-major: out[px, co] = sum_taps xpad[ci, px+s].T @ w_tap[ci, co]
     with lhsT (stationary) = x tile [128ci x 116px] (2 padded rows), rhs =
     w tap [128ci x 256co], PSUM f32 [116 x 256].  All values are +-1 in
     bf16, accumulation is f32 -> arithmetic is exact.
  5. DVE copies PSUM -> SBUF stage; two large DMAs per half-image write the
     NHWC output (even rows / odd rows) back to HBM.

Built on bacc.Bacc (not raw Bass) so multi-semaphore waits are legalized
into EventSemaphore chains (TRN2 instructions hold at most one sync wait).
"""

import sys

if "/opt/trn_rl_repo" not in sys.path:
    sys.path.insert(0, "/opt/trn_rl_repo")

import numpy as np

import concourse.bacc as bacc
import concourse.bass as bass
import concourse.mybir as mybir
from concourse.tile import TileContext
from concourse.bass_utils import run_bass_kernel_spmd

N_CORES = 8
IMGS = 8  # images per core
H = W = 56
C = 128  # input channels (= contraction dim = SBUF partitions)
O = 256  # output channels
PW = 58  # padded row width
PH = 58  # padded rows per image (rows 0 and 57 are the SAME-padding rows)
PPI = PH * PW  # padded pixels per image (3364)
GUARD_L = 1  # zero guard before image 0 (tap offset -59 at tile 0)
GUARD_R = 4
TILES = H // 2  # 28 output tiles per image, 2 output rows each
F32 = mybir.dt.float32
BF16 = mybir.dt.bfloat16
U16 = mybir.dt.uint16

# tap order k = 3*di + dj ; shift in padded flat coords
TAP_SHIFTS = [PW * (di - 1) + (dj - 1) for di in range(3) for dj in range(3)]


def build_nc() -> bass.Bass:
    nc = bacc.Bacc()
    x_t = nc.dram_tensor("x", [IMGS, H, W, C], F32, kind="ExternalInput")
    # host-binarized weights, laid out [tap*co, ci] so one xbar DMA-transpose
    # loads them as [ci, tap*co]
    wbt_t = nc.dram_tensor("wbt", [9 * O, C], BF16, kind="ExternalInput")
    y_t = nc.dram_tensor("out", [IMGS, H, W, O], F32, kind="ExternalOutput")
    # per-pair bf16 bounce tensors keep DRAM dependency tracking precise
    xb_ts = [
        nc.dram_tensor(f"xb{p}", [2 * H * W, C], BF16) for p in range(IMGS // 2)
    ]

    with TileContext(nc) as tc:
        with (
            tc.tile_pool(name="const", bufs=1) as constp,
            tc.tile_pool(name="xtr", bufs=IMGS // 2) as xtrp,
            tc.tile_pool(name="stage", bufs=3) as stagep,
            tc.tile_pool(name="psum", bufs=6, space="PSUM") as psump,
        ):
            # ---- weights: single xbar transpose load of host-binarized w ----
            wb = constp.tile([C, 9 * O], BF16)
            nc.sync.dma_start(out=wb[:], in_=wbt_t[:], transpose=True)

            # ---- per-image zero-padded, channel-major input planes ----
            # Zero only the padding ranges (disjoint from the binarize write
            # range) to keep the dependency structure lean.
            xpads = []
            for i in range(IMGS):
                xp = constp.tile([C, GUARD_L + PPI + GUARD_R], BF16, tag=f"xpad{i}")
                # head: guard + top pad row + col0 of data row 1 -> [0, 60)
                nc.vector.memset(xp[:, 0:60], 0.0)
                # interior: col57 of row r + col0 of row r+1 -> [58k, 58k+2)
                nc.vector.memset(
                    xp[:, 116 : 116 + 55 * PW].rearrange("c (r w) -> c r w", w=PW)[
                        :, :, 0:2
                    ],
                    0.0,
                )
                # tail: col57 of row 56 + bottom pad row + guard
                nc.vector.memset(xp[:, 3306 : GUARD_L + PPI + GUARD_R], 0.0)
                xpads.append(xp)

            # ---- input pipeline: cast pairs, transpose pairs ----
            xtrs = {}
            for p in range(IMGS // 2):
                nc.gpsimd.dma_start(
                    out=xb_ts[p][:],
                    in_=x_t[2 * p : 2 * p + 2].rearrange("n h w c -> (n h w) c"),
                )
                xtr = xtrp.tile([C, 2 * H * W], BF16)
                nc.sync.dma_start(out=xtr[:], in_=xb_ts[p][:], transpose=True)
                xtrs[p] = xtr

            for i in range(IMGS):
                xtr = xtrs[i // 2]
                xoff = (i % 2) * H * W
                # binarize + scatter into padded rows (56 rows, stride 58)
                s0 = GUARD_L + PW + 1
                dst = xpads[i][:, s0 : s0 + H * PW].rearrange(
                    "c (r w) -> c r w", w=PW
                )[:, :, 0:W]
                src = xtr[:, xoff : xoff + H * W].rearrange("c (r w) -> c r w", w=W)
                nc.vector.tensor_scalar(
                    dst.bitcast(U16),
                    src.bitcast(U16),
                    0x8000,
                    0x3F80,
                    op0=mybir.AluOpType.bitwise_and,
                    op1=mybir.AluOpType.bitwise_or,
                )

                # ---- 28 output tiles (2 rows each) of 9 accumulating matmuls,
                # staged in half-image chunks of 14 tiles to bound SBUF ----
                HT = TILES // 2  # 14
                for half in range(2):
                    stage = stagep.tile([128, HT * O], F32)
                    st3 = stage[:].rearrange("p (t o) -> p t o", o=O)
                    for th in range(HT):
                        t = half * HT + th
                        ps = psump.tile([128, O], F32)
                        p0 = GUARD_L + PW * (1 + 2 * t)  # padded start of tile
                        for k, s in enumerate(TAP_SHIFTS):
                            a = p0 + s
                            nc.tensor.matmul(
                                ps[:116, :],
                                xpads[i][:, a : a + 116],
                                wb[:, k * O : (k + 1) * O],
                                start=(k == 0),
                                stop=(k == 8),
                            )
                        nc.vector.tensor_copy(
                            stage[:116, th * O : (th + 1) * O], ps[:116, :]
                        )

                    # ---- write out: partitions 1..56 = even rows, 59..114 odd
                    rows = y_t[i][half * 2 * HT : (half + 1) * 2 * HT]
                    ye = rows.rearrange("(r2 two) w c -> two w r2 c", two=2)
                    nc.gpsimd.dma_start(out=ye[0], in_=st3[1 : 1 + W])
                    nc.gpsimd.dma_start(out=ye[1], in_=st3[59 : 59 + W])

    nc.finalize()
    return nc


_NC_CACHE = None


def _get_nc():
    global _NC_CACHE
    if _NC_CACHE is None:
        _NC_CACHE = build_nc()
    return _NC_CACHE


def prep_wbt(w: np.ndarray) -> np.ndarray:
    """Binarize + transpose weights on host: (3,3,128,256) f32 ->
    [9*256, 128] bf16 with exact +-1 values (replicated to every core)."""
    import ml_dtypes

    wb = np.where(w >= 0, np.float32(1.0), np.float32(-1.0))
    # [di, dj, ci, co] -> [(di dj) co, ci]
    wbt = wb.transpose(0, 1, 3, 2).reshape(9 * O, C)
    return np.ascontiguousarray(wbt.astype(ml_dtypes.bfloat16))


def _ntff_hook():
    """NTFF capture context manager via the axon PJRT .so (the installed
    antenv lacks axon_hooks, so build the ctypes hook directly)."""
    sys.path.insert(0, "/root/.axon_site")
    from trn_agent_boot.trn_boot import _ntff_profile_via_ctypes

    return _ntff_profile_via_ctypes("/opt/axon/libaxon_pjrt.so")


def run(inputs: dict, profile_dir: str | None = None):
    """Run on all 8 NeuronCores. Returns (full_output, BassKernelResults)."""
    x = np.ascontiguousarray(np.asarray(inputs["x"], dtype=np.float32))
    w = np.ascontiguousarray(np.asarray(inputs["w"], dtype=np.float32))
    assert x.shape == (N_CORES * IMGS, H, W, C), x.shape
    assert w.shape == (3, 3, C, O), w.shape

    nc = _get_nc()
    wbt = prep_wbt(w)
    in_maps = [
        {"x": x[i * IMGS : (i + 1) * IMGS], "wbt": wbt} for i in range(N_CORES)
    ]
    if profile_dir is not None:
        hook = _ntff_hook()
        with hook(profile_dir, [0]):
            res = run_bass_kernel_spmd(nc, in_maps, list(range(N_CORES)))
    else:
        res = run_bass_kernel_spmd(nc, in_maps, list(range(N_CORES)))
    out = np.concatenate([res.results[i]["out"] for i in range(N_CORES)], axis=0)
    return out, res


def kernel(**inputs: np.ndarray) -> np.ndarray:
    out, _ = run(inputs)
    return out

